# revision 7
# baseline (speedup 1.0000x reference)
"""Causal bilinear self-attention kernel for 8 Trainium2 NeuronCores.

Sharding: core c handles batch b = c//4 and head group g = c%4 (4 of 16
heads, feature slice [256g, 256g+256)).  Each core computes its partial
output-projection contribution y_partial = z_slice @ Wproj[:, slice].T and
the host sums the 4 partials per batch.

The whole PE path runs in bfloat16 (x, weights, roped q/k, v, pattern, z,
y partials); PSUM accumulation stays fp32.  RoPE is applied on-chip: the
projection PSUM tile is drained to SBUF (Act), the sin-product of the
partner rows is formed by 4 partition-offset muls against a source-aligned
signed sin table (Pool), the cos-product on DVE, and the final add on Pool
writes the bf16 roped tile.  This removes the 4 sign-permuted-weight
projections entirely (~131k PE rows).

Schedule: phase A t-half 0 runs the q projection kc-major (4 concurrent
PSUM accumulators) so PE can start as soon as the first x tile lands;
t-half 1 is a generator interleaved into phase B.  Phase B blocks compute
scores S, S2 per head pair (2 row-tile_position matmuls each), the masked
bilinear product (Act copy + Pool mask + DVE mul), and accumulate z.  The
two independent blocks B(2,1)/B(3,0) are zipped to fill each other's
latency stalls.  Output projection C(qq) is interleaved per q-block.
"""

import numpy as np
import ml_dtypes

import concourse.tile as tile
from concourse import bacc, mybir
from concourse.bass_utils import run_bass_kernel_spmd

D_MODEL = 1024
N_HEAD = 16
HEAD_DIM = 64  # Dh
B, T = 2, 2048
ROPE_BASE = 10000.0
N_CORES = 8
HG = 4          # heads per core
FS = HG * HEAD_DIM  # 256 features per core

F32 = mybir.dt.float32
BF16 = mybir.dt.bfloat16
NP_BF16 = ml_dtypes.bfloat16

# (dst_row_start, src_row_start) pairs of the 32-row rope partner shuffle
SHUF = ((0, 32), (32, 0), (64, 96), (96, 64))

_PROGRAM = None


def _build_program():
    nc = bacc.Bacc("TRN2", target_bir_lowering=False, debug=False)

    xt_d = nc.dram_tensor("xt", [D_MODEL, T], BF16, kind="ExternalInput").ap()
    wpk_d = nc.dram_tensor("wpk", [5, 128, 2048], BF16, kind="ExternalInput").ap()
    wpj_d = nc.dram_tensor("wpj", [128, 2048], BF16, kind="ExternalInput").ap()
    tabs_d = nc.dram_tensor("tabs", [2, 128, 2048], BF16, kind="ExternalInput").ap()
    masks_d = nc.dram_tensor("masks", [128, 2048], F32, kind="ExternalInput").ap()
    y_d = nc.dram_tensor("y", [T, D_MODEL], BF16, kind="ExternalOutput").ap()

    with tile.TileContext(nc) as tc:
        with (
            tc.tile_pool(name="pers", bufs=1) as pers,
            tc.tile_pool(name="xp", bufs=16) as xp,
            tc.tile_pool(name="wp", bufs=5) as wp,
            tc.tile_pool(name="mkp", bufs=1) as mkp,
            tc.tile_pool(name="qsp", bufs=6) as qsp,
            tc.tile_pool(name="qcp", bufs=6) as qcp,
            tc.tile_pool(name="tmp", bufs=6) as tmp,
            tc.tile_pool(name="tsp", bufs=4) as tsp,
            tc.tile_pool(name="ptp", bufs=4) as ptp,
            tc.tile_pool(name="ysb", bufs=4) as ysb,
            tc.tile_pool(name="psA", bufs=2, space="PSUM") as psA,
            tc.tile_pool(name="psS", bufs=2, space="PSUM") as psS,
            tc.tile_pool(name="psS2", bufs=2, space="PSUM") as psS2,
            tc.tile_pool(name="psZ", bufs=2, space="PSUM") as psZ,
        ):
            # persistent tiles
            proj = [pers.tile([128, T], BF16, tag=f"proj{i}", name=f"proj{i}")
                    for i in range(8)]
            # proj[2*ti+f] = chunk f of tensor ti (0=q,1=k,2=q2,3=k2)
            vt = [pers.tile([128, 1024], BF16, tag=f"v{i}", name=f"v{i}")
                  for i in range(4)]
            zt = [pers.tile([128, T], BF16, tag=f"z{i}", name=f"z{i}")
                  for i in range(2)]
            ctab = pers.tile([128, 2048], BF16, tag="ctab")
            stabx = pers.tile([128, 2048], BF16, tag="stabx")
            masks = mkp.tile([128, 2048], F32, tag="masks")
            wpjt = pers.tile([128, 2048], BF16, tag="wpjt")

            wts = []       # 5 resident weight tiles (q,k,q2,k2,v)
            xth = [[], []]  # x tiles per t-half

            def rope_consume(ps, ti, f, col):
                """Drain + RoPE a [128,512] projection PSUM tile into
                proj[2*ti+f][:, col:col+512]."""
                qs = qsp.tile([128, 512], BF16, tag="qs", name="qs")
                nc.scalar.copy(qs[:], ps[:])
                qc = qcp.tile([128, 512], BF16, tag="qc", name="qc")
                nc.gpsimd.tensor_mul(qc[:], qs[:], ctab[:, col:col + 512])
                tm = tmp.tile([128, 512], BF16, tag="tm", name="tm")
                for ds, ss in SHUF:
                    nc.vector.tensor_mul(
                        tm[ds:ds + 32, :], qs[ss:ss + 32, :],
                        stabx[ss:ss + 32, col:col + 512],
                    )
                nc.gpsimd.tensor_add(
                    proj[2 * ti + f][:, col:col + 512], qc[:], tm[:]
                )

            def qk_unit(wt, ti, f, tt, th):
                """One standard projection unit (8-deep kc chain)."""
                ps = psA.tile([128, 512], F32, tag="psa", name="psa")
                for kc in range(8):
                    nc.tensor.matmul(
                        ps[:],
                        wt[:, kc * 256 + f * 128: kc * 256 + f * 128 + 128],
                        xth[th][kc][:, tt * 512: tt * 512 + 512],
                        start=(kc == 0),
                        stop=(kc == 7),
                    )
                rope_consume(ps, ti, f, th * 1024 + tt * 512)

            def v_unit(wt, m, th):
                tg = th * 8 + m
                psv = psA.tile([128, 512], F32, tag="psa", name="psa")
                for kc in range(8):
                    nc.tensor.matmul(
                        psv[:, 0:256],
                        xth[th][kc][:, m * 128: m * 128 + 128],
                        wt[:, kc * 256: kc * 256 + 256],
                        start=(kc == 0),
                        stop=(kc == 7),
                    )
                nc.scalar.copy(
                    vt[tg // 4][:, (tg % 4) * 256: (tg % 4) * 256 + 256],
                    psv[:, 0:256],
                )

            def emit_A0():
                # q weights first, then x t-half 0, tabs, remaining weights
                wq = wp.tile([128, 2048], BF16, tag="wt", name="wt")
                nc.sync.dma_start(wq[:, 0:1024], wpk_d[0][:, 0:1024])
                wts.append(wq)
                for kc in range(8):
                    xtl = xp.tile([128, 1024], BF16, tag="xt", name="xtl")
                    nc.sync.dma_start(
                        xtl[:], xt_d[kc * 128: kc * 128 + 128, 0:1024]
                    )
                    xth[0].append(xtl)
                    if kc == 0:
                        nc.sync.dma_start(wq[:, 1024:2048], wpk_d[0][:, 1024:2048])
                for wi in range(1, 5):
                    wt = wp.tile([128, 2048], BF16, tag="wt", name="wt")
                    nc.sync.dma_start(wt[:], wpk_d[wi])
                    wts.append(wt)
                nc.sync.dma_start(ctab[:], tabs_d[0])
                nc.sync.dma_start(stabx[:], tabs_d[1])
                nc.sync.dma_start(masks[:], masks_d)

                # kc-major q projection: 4 concurrent accumulators so the PE
                # starts on x tile 0 instead of waiting for the full t-half
                qpools = [(psA, "psa"), (psS, "sps"), (psS2, "s2ps"), (psZ, "zps")]
                units = [(0, 0), (1, 0), (0, 1), (1, 1)]  # (f, tt)
                qps = [pool.tile([128, 512], F32, tag=tag, name="qps")
                       for pool, tag in qpools]
                for kc in range(8):
                    for u, (f, tt) in enumerate(units):
                        nc.tensor.matmul(
                            qps[u][:],
                            wq[:, kc * 256 + f * 128: kc * 256 + f * 128 + 128],
                            xth[0][kc][:, tt * 512: tt * 512 + 512],
                            start=(kc == 0),
                            stop=(kc == 7),
                        )
                # (tt, f)-major so each B block's proj dependencies clear the
                # Pool/Act queues in block order
                for u, (f, tt) in enumerate(units):
                    rope_consume(qps[u], 0, f, tt * 512)
                    for ti in range(1, 4):
                        qk_unit(wts[ti], ti, f, tt, 0)
                for m in range(8):
                    v_unit(wts[4], m, 0)

                # queue t-half-1 x and the output weight behind phase-A DMA
                for kc in range(8):
                    xtl = xp.tile([128, 1024], BF16, tag="xt", name="xtl")
                    nc.sync.dma_start(
                        xtl[:], xt_d[kc * 128: kc * 128 + 128, 1024:2048]
                    )
                    xth[1].append(xtl)
                nc.sync.dma_start(wpjt[:], wpj_d[:])

            def qk_unit_gen(wt, ti, f, tt, th):
                """qk_unit split at op granularity so interleaving into B
                blocks never head-of-line-blocks a latency-critical op."""
                ps = psA.tile([128, 512], F32, tag="psa", name="psa")
                for kc in range(8):
                    nc.tensor.matmul(
                        ps[:],
                        wt[:, kc * 256 + f * 128: kc * 256 + f * 128 + 128],
                        xth[th][kc][:, tt * 512: tt * 512 + 512],
                        start=(kc == 0),
                        stop=(kc == 7),
                    )
                yield
                col = th * 1024 + tt * 512
                qs = qsp.tile([128, 512], BF16, tag="qs", name="qs")
                nc.scalar.copy(qs[:], ps[:])
                yield
                qc = qcp.tile([128, 512], BF16, tag="qc", name="qc")
                nc.gpsimd.tensor_mul(qc[:], qs[:], ctab[:, col:col + 512])
                yield
                tm = tmp.tile([128, 512], BF16, tag="tm", name="tm")
                for ds, ss in SHUF:
                    nc.vector.tensor_mul(
                        tm[ds:ds + 32, :], qs[ss:ss + 32, :],
                        stabx[ss:ss + 32, col:col + 512],
                    )
                    yield
                nc.gpsimd.tensor_add(
                    proj[2 * ti + f][:, col:col + 512], qc[:], tm[:]
                )
                yield

            def v_unit_gen(wt, m, th):
                tg = th * 8 + m
                psv = psA.tile([128, 512], F32, tag="psa", name="psa")
                for kc in range(8):
                    nc.tensor.matmul(
                        psv[:, 0:256],
                        xth[th][kc][:, m * 128: m * 128 + 128],
                        wt[:, kc * 256: kc * 256 + 256],
                        start=(kc == 0),
                        stop=(kc == 7),
                    )
                yield
                nc.scalar.copy(
                    vt[tg // 4][:, (tg % 4) * 256: (tg % 4) * 256 + 256],
                    psv[:, 0:256],
                )
                yield

            def emit_A1():
                for tt in range(2):
                    for f in range(2):
                        for ti in range(4):
                            yield from qk_unit_gen(wts[ti], ti, f, tt, 1)
                    if tt == 0:
                        for m in range(4):
                            yield from v_unit_gen(wts[4], m, 1)
                for m in range(4, 8):
                    yield from v_unit_gen(wts[4], m, 1)

            def drain(gen, n):
                for _ in range(n):
                    try:
                        next(gen)
                    except StopIteration:
                        return False
                return True

            def scores_kk(qq, hp, kk):
                off = max(0, kk - 4 * qq) * 128
                qsl = slice(qq * 512 + off, qq * 512 + 512)
                ksl = slice(kk * 128, kk * 128 + 128)
                kT, qT = proj[2 + hp], proj[0 + hp]
                k2T, q2T = proj[6 + hp], proj[4 + hp]
                sps = [None, None]
                s2ps = [None, None]
                for hh in range(2):
                    rb = 64 * hh
                    sp = psS.tile([128, 512], F32, tag="sps", name="sps")
                    nc.tensor.matmul(
                        sp[:, off:512], kT[rb:rb + 64, ksl], qT[rb:rb + 64, qsl],
                        start=True, stop=True, tile_position=(rb, 0),
                    )
                    sps[hh] = sp
                for hh in range(2):
                    rb = 64 * hh
                    s2 = psS2.tile([128, 512], F32, tag="s2ps", name="s2ps")
                    nc.tensor.matmul(
                        s2[:, off:512], k2T[rb:rb + 64, ksl], q2T[rb:rb + 64, qsl],
                        start=True, stop=True, tile_position=(rb, 0),
                    )
                    s2ps[hh] = s2
                return sps, s2ps

            def bilinear_z(qq, hp, kk, sps, s2ps, zps, last):
                off = max(0, kk - 4 * qq) * 128
                for hh in range(2):
                    ts = tsp.tile([128, 512], F32, tag="ts", name="ts")
                    nc.scalar.copy(ts[:, off:512], sps[hh][:, off:512])
                    if kk >= 4 * qq:
                        j = kk - 4 * qq
                        nc.gpsimd.tensor_mul(
                            ts[:, off:off + 128],
                            ts[:, off:off + 128],
                            masks[:, j * 512 + off: j * 512 + off + 128],
                        )
                    pt = ptp.tile([128, 512], BF16, tag="pt", name="pt")
                    nc.vector.tensor_mul(
                        pt[:, off:512], ts[:, off:512], s2ps[hh][:, off:512]
                    )
                    nc.tensor.matmul(
                        zps[64 * hh: 64 * hh + 64, off:512],
                        vt[kk // 4][:, (kk % 4) * 256 + (2 * hp + hh) * 64
                                    : (kk % 4) * 256 + (2 * hp + hh) * 64 + 64],
                        pt[:, off:512],
                        start=(kk == 0),
                        stop=(kk == last),
                    )

            def drain_z(qq, hp, zps):
                nc.scalar.copy(
                    zt[hp][:, qq * 512: qq * 512 + 512], zps[:]
                )

            def emit_B_gen(qq, hp, zpool, ztag):
                """Per-kk generator; zps holds both heads (hh at rows 64*hh)."""
                zps = zpool.tile([128, 512], F32, tag=ztag, name="zpg")
                last = 4 * qq + 3
                for kk in range(last + 1):
                    sps, s2ps = scores_kk(qq, hp, kk)
                    bilinear_z(qq, hp, kk, sps, s2ps, zps, last)
                    yield
                drain_z(qq, hp, zps)

            def emit_C_step(qq, ypool, ytag, tg, oo):
                yps = ypool.tile([128, 512], F32, tag=ytag, name="yps")
                for ci in range(2):
                    nc.tensor.matmul(
                        yps[:],
                        zt[ci][:, tg * 128: tg * 128 + 128],
                        wpjt[:, ci * 1024 + oo * 512
                             : ci * 1024 + oo * 512 + 512],
                        start=(ci == 0),
                        stop=(ci == 1),
                    )
                yo = ysb.tile([128, 512], BF16, tag="yo", name="yo")
                if (2 * tg + oo) % 2 == 0:
                    nc.scalar.copy(yo[:], yps[:])
                else:
                    nc.vector.tensor_copy(yo[:], yps[:])
                nc.sync.dma_start(
                    y_d[tg * 128: tg * 128 + 128, oo * 512: oo * 512 + 512],
                    yo[:],
                )

            def emit_C(qq, ypool, ytag):
                for tg in range(4 * qq, 4 * qq + 4):
                    for oo in range(2):
                        emit_C_step(qq, ypool, ytag, tg, oo)

            def emit_C_gen(qq, ypool, ytag):
                for tg in range(4 * qq, 4 * qq + 4):
                    for oo in range(2):
                        emit_C_step(qq, ypool, ytag, tg, oo)
                        yield

            # ---------------- emission schedule ----------------
            emit_A0()
            gen1 = emit_A1()

            def zip_blocks(qq, drip, extra=None, extra_every=0):
                ga = emit_B_gen(qq, 0, psZ, "zps")
                gb = emit_B_gen(qq, 1, psZ, "zps")
                rnd = 0
                alive = [ga, gb]
                while alive:
                    for g in list(alive):
                        try:
                            next(g)
                        except StopIteration:
                            alive.remove(g)
                    if drip:
                        drain(gen1, drip)
                    if extra is not None and extra_every and rnd % extra_every == extra_every - 1:
                        drain(extra, 1)
                    rnd += 1

            for qq in range(2):
                zip_blocks(qq, drip=3)
                for tg in range(4 * qq, 4 * qq + 4):
                    for oo in range(2):
                        emit_C_step(qq, psA, "psa", tg, oo)
                        drain(gen1, 2)
            zip_blocks(2, drip=4)
            drain(gen1, 1000)  # flush remaining A(1) units
            gc2 = emit_C_gen(2, psA, "psa")
            zip_blocks(3, drip=0, extra=gc2, extra_every=2)
            drain(gc2, 1000)
            emit_C(3, psA, "psa")

    nc.compile()
    return nc


def _get_program():
    global _PROGRAM
    if _PROGRAM is None:
        _PROGRAM = _build_program()
    return _PROGRAM


def _pack_w(ws):
    """ws: [256, 1024] (out-feature rows, in-feature cols) ->
    packed [128, 2048] where chunk kc, half f lives at cols
    [kc*256 + f*128, ...+128): lhsT tile = ws.T[kc*128:(kc+1)*128, f*128:...]."""
    a = np.ascontiguousarray(ws.T)  # [1024, 256]
    return np.ascontiguousarray(
        a.reshape(8, 128, 256).transpose(1, 0, 2).reshape(128, 2048)
    )


def _make_tabs():
    inv = 1.0 / (ROPE_BASE ** (np.arange(0, HEAD_DIM, 2, dtype=np.float32) / HEAD_DIM))
    t = np.arange(T, dtype=np.float32)
    ang = np.outer(t, inv)  # [T, 32]
    c32 = np.cos(ang).astype(np.float32).T  # [32, T]
    s32 = np.sin(ang).astype(np.float32).T
    ctab = np.tile(c32, (4, 1))  # [128, T]
    # source-aligned signed sin table: the shuffle op reads src rows ss:ss+32
    # of both the drained q tile and this table, writing dst rows ds:ds+32.
    # dst 0:32 needs +sin (src rows 32:64), dst 32:64 needs -sin (src 0:32).
    stabx = np.concatenate([-s32, s32, -s32, s32], axis=0)  # [128, T]
    tabs = np.ascontiguousarray(np.stack([ctab, stabx])).astype(NP_BF16)
    r = np.arange(128)[:, None]
    ccol = np.arange(512)[None, :]
    masks = np.ascontiguousarray(np.concatenate(
        [(ccol >= r + 128 * j).astype(np.float32) for j in range(4)], axis=1
    ))  # [128, 2048]
    return tabs, masks


def kernel(x, Wq, Wk, Wq2, Wk2, Wv, Wproj):
    x = np.asarray(x, dtype=np.float32)
    Wq = np.asarray(Wq, dtype=np.float32)
    Wk = np.asarray(Wk, dtype=np.float32)
    Wq2 = np.asarray(Wq2, dtype=np.float32)
    Wk2 = np.asarray(Wk2, dtype=np.float32)
    Wv = np.asarray(Wv, dtype=np.float32)
    Wproj = np.asarray(Wproj, dtype=np.float32)

    nc = _get_program()
    tabs, masks = _make_tabs()

    in_maps = []
    for c in range(N_CORES):
        b, g = divmod(c, HG)
        fsl = slice(g * FS, g * FS + FS)
        wpk = np.ascontiguousarray(
            np.stack(
                [
                    _pack_w(Wq[fsl] * (1.0 / HEAD_DIM)),
                    _pack_w(Wk[fsl]),
                    _pack_w(Wq2[fsl] * (1.0 / HEAD_DIM)),
                    _pack_w(Wk2[fsl]),
                    _pack_w(Wv[fsl]),
                ]
            )
        ).astype(NP_BF16)
        wpj = np.ascontiguousarray(
            Wproj[:, fsl].T.reshape(2, 128, 1024).transpose(1, 0, 2).reshape(128, 2048)
        ).astype(NP_BF16)
        xt = np.ascontiguousarray(x[b].T).astype(NP_BF16)
        in_maps.append({"xt": xt, "wpk": wpk, "wpj": wpj, "tabs": tabs, "masks": masks})

    res = run_bass_kernel_spmd(nc, in_maps, list(range(N_CORES))).results

    y = np.zeros((B, T, D_MODEL), dtype=np.float64)
    for c in range(N_CORES):
        b = c // HG
        y[b] += np.asarray(res[c]["y"]).astype(np.float64)
    return y.astype(np.float32)


# revision 8
# speedup vs baseline: 1.0375x; 1.0375x over previous
"""Causal bilinear self-attention kernel for 8 Trainium2 NeuronCores.

Sharding: core c handles batch b = c//4 and head group g = c%4 (4 of 16
heads, feature slice [256g, 256g+256)).  Each core computes its partial
output-projection contribution y_partial = z_slice @ Wproj[:, slice].T and
the host sums the 4 partials per batch.

The whole PE path runs in bfloat16 (x, weights, roped q/k, v, pattern, z,
y partials); PSUM accumulation stays fp32.  RoPE is applied on-chip: the
projection PSUM tile is drained to SBUF (Act), the sin-product of the
partner rows is formed by 4 partition-offset muls against a source-aligned
signed sin table (Pool), the cos-product on DVE, and the final add on Pool
writes the bf16 roped tile.  This removes the 4 sign-permuted-weight
projections entirely (~131k PE rows).

Schedule: phase A t-half 0 runs the q projection kc-major (4 concurrent
PSUM accumulators) so PE can start as soon as the first x tile lands;
t-half 1 is a generator interleaved into phase B.  Phase B blocks compute
scores S, S2 per head pair (2 row-tile_position matmuls each), the masked
bilinear product (Act copy + Pool mask + DVE mul), and accumulate z.  The
two independent blocks B(2,1)/B(3,0) are zipped to fill each other's
latency stalls.  Output projection C(qq) is interleaved per q-block.
"""

import numpy as np
import ml_dtypes

import concourse.tile as tile
from concourse import bacc, mybir
from concourse.bass_utils import run_bass_kernel_spmd

D_MODEL = 1024
N_HEAD = 16
HEAD_DIM = 64  # Dh
B, T = 2, 2048
ROPE_BASE = 10000.0
N_CORES = 8
HG = 4          # heads per core
FS = HG * HEAD_DIM  # 256 features per core

F32 = mybir.dt.float32
BF16 = mybir.dt.bfloat16
NP_BF16 = ml_dtypes.bfloat16

# (dst_row_start, src_row_start) pairs of the 32-row rope partner shuffle
SHUF = ((0, 32), (32, 0), (64, 96), (96, 64))

_PROGRAM = None


def _build_program():
    nc = bacc.Bacc("TRN2", target_bir_lowering=False, debug=False)

    xt_d = nc.dram_tensor("xt", [D_MODEL, T], BF16, kind="ExternalInput").ap()
    wpk_d = nc.dram_tensor("wpk", [5, 128, 2048], BF16, kind="ExternalInput").ap()
    wpj_d = nc.dram_tensor("wpj", [128, 2048], BF16, kind="ExternalInput").ap()
    tabs_d = nc.dram_tensor("tabs", [2, 128, 2048], BF16, kind="ExternalInput").ap()
    masks_d = nc.dram_tensor("masks", [128, 2048], F32, kind="ExternalInput").ap()
    y_d = nc.dram_tensor("y", [T, D_MODEL], BF16, kind="ExternalOutput").ap()

    with tile.TileContext(nc) as tc:
        with (
            tc.tile_pool(name="pers", bufs=1) as pers,
            tc.tile_pool(name="xp", bufs=16) as xp,
            tc.tile_pool(name="wp", bufs=5) as wp,
            tc.tile_pool(name="mkp", bufs=1) as mkp,
            tc.tile_pool(name="qsp", bufs=6) as qsp,
            tc.tile_pool(name="qcp", bufs=6) as qcp,
            tc.tile_pool(name="tmp", bufs=6) as tmp,
            tc.tile_pool(name="tsp", bufs=4) as tsp,
            tc.tile_pool(name="ptp", bufs=4) as ptp,
            tc.tile_pool(name="ysb", bufs=8) as ysb,
            tc.tile_pool(name="psA", bufs=2, space="PSUM") as psA,
            tc.tile_pool(name="psS", bufs=2, space="PSUM") as psS,
            tc.tile_pool(name="psS2", bufs=2, space="PSUM") as psS2,
            tc.tile_pool(name="psZ", bufs=2, space="PSUM") as psZ,
        ):
            # persistent tiles
            proj = [pers.tile([128, T], BF16, tag=f"proj{i}", name=f"proj{i}")
                    for i in range(8)]
            # proj[2*ti+f] = chunk f of tensor ti (0=q,1=k,2=q2,3=k2)
            vt = [pers.tile([128, 1024], BF16, tag=f"v{i}", name=f"v{i}")
                  for i in range(4)]
            zt = [pers.tile([128, T], BF16, tag=f"z{i}", name=f"z{i}")
                  for i in range(2)]
            ctab = pers.tile([128, 2048], BF16, tag="ctab")
            stabx = pers.tile([128, 2048], BF16, tag="stabx")
            masks = mkp.tile([128, 2048], F32, tag="masks")
            wpjt = pers.tile([128, 2048], BF16, tag="wpjt")

            wts = []       # 5 resident weight tiles (q,k,q2,k2,v)
            xth = [[], []]  # x tiles per t-half

            rope_flip = [0]

            def rope_consume(ps, ti, f, col):
                """Drain + RoPE a [128,512] projection PSUM tile into
                proj[2*ti+f][:, col:col+512].  The cos-mul and final add
                alternate between Pool and DVE so neither queue backs up."""
                qs = qsp.tile([128, 512], BF16, tag="qs", name="qs")
                nc.scalar.copy(qs[:], ps[:])
                eng = nc.gpsimd if rope_flip[0] % 2 == 0 else nc.vector
                rope_flip[0] += 1
                qc = qcp.tile([128, 512], BF16, tag="qc", name="qc")
                eng.tensor_mul(qc[:], qs[:], ctab[:, col:col + 512])
                tm = tmp.tile([128, 512], BF16, tag="tm", name="tm")
                for ds, ss in SHUF:
                    nc.vector.tensor_mul(
                        tm[ds:ds + 32, :], qs[ss:ss + 32, :],
                        stabx[ss:ss + 32, col:col + 512],
                    )
                eng.tensor_add(
                    proj[2 * ti + f][:, col:col + 512], qc[:], tm[:]
                )

            def qk_unit(wt, ti, f, tt, th):
                """One standard projection unit (8-deep kc chain)."""
                ps = psA.tile([128, 512], F32, tag="psa", name="psa")
                for kc in range(8):
                    nc.tensor.matmul(
                        ps[:],
                        wt[:, kc * 256 + f * 128: kc * 256 + f * 128 + 128],
                        xth[th][kc][:, tt * 512: tt * 512 + 512],
                        start=(kc == 0),
                        stop=(kc == 7),
                    )
                rope_consume(ps, ti, f, th * 1024 + tt * 512)

            def v_unit(wt, m, th):
                tg = th * 8 + m
                psv = psA.tile([128, 512], F32, tag="psa", name="psa")
                for kc in range(8):
                    nc.tensor.matmul(
                        psv[:, 0:256],
                        xth[th][kc][:, m * 128: m * 128 + 128],
                        wt[:, kc * 256: kc * 256 + 256],
                        start=(kc == 0),
                        stop=(kc == 7),
                    )
                nc.scalar.copy(
                    vt[tg // 4][:, (tg % 4) * 256: (tg % 4) * 256 + 256],
                    psv[:, 0:256],
                )

            def emit_A0():
                # q weights first, then x t-half 0, tabs, remaining weights
                wq = wp.tile([128, 2048], BF16, tag="wt", name="wt")
                nc.sync.dma_start(wq[:, 0:1024], wpk_d[0][:, 0:1024])
                wts.append(wq)
                for kc in range(8):
                    xtl = xp.tile([128, 1024], BF16, tag="xt", name="xtl")
                    nc.sync.dma_start(
                        xtl[:], xt_d[kc * 128: kc * 128 + 128, 0:1024]
                    )
                    xth[0].append(xtl)
                    if kc == 0:
                        nc.sync.dma_start(wq[:, 1024:2048], wpk_d[0][:, 1024:2048])
                for wi in range(1, 5):
                    wt = wp.tile([128, 2048], BF16, tag="wt", name="wt")
                    nc.sync.dma_start(wt[:], wpk_d[wi])
                    wts.append(wt)
                nc.sync.dma_start(ctab[:], tabs_d[0])
                nc.sync.dma_start(stabx[:], tabs_d[1])
                nc.sync.dma_start(masks[:], masks_d)

                # kc-major q projection: 4 concurrent accumulators so the PE
                # starts on x tile 0 instead of waiting for the full t-half
                qpools = [(psA, "psa"), (psS, "sps"), (psS2, "s2ps"), (psZ, "zps")]
                units = [(0, 0), (1, 0), (0, 1), (1, 1)]  # (f, tt)
                qps = [pool.tile([128, 512], F32, tag=tag, name="qps")
                       for pool, tag in qpools]
                for kc in range(8):
                    for u, (f, tt) in enumerate(units):
                        nc.tensor.matmul(
                            qps[u][:],
                            wq[:, kc * 256 + f * 128: kc * 256 + f * 128 + 128],
                            xth[0][kc][:, tt * 512: tt * 512 + 512],
                            start=(kc == 0),
                            stop=(kc == 7),
                        )
                # (tt, f)-major so each B block's proj dependencies clear the
                # Pool/Act queues in block order
                for u, (f, tt) in enumerate(units):
                    rope_consume(qps[u], 0, f, tt * 512)
                    for ti in range(1, 4):
                        qk_unit(wts[ti], ti, f, tt, 0)
                for m in range(8):
                    v_unit(wts[4], m, 0)

                # queue t-half-1 x and the output weight behind phase-A DMA
                for kc in range(8):
                    xtl = xp.tile([128, 1024], BF16, tag="xt", name="xtl")
                    nc.sync.dma_start(
                        xtl[:], xt_d[kc * 128: kc * 128 + 128, 1024:2048]
                    )
                    xth[1].append(xtl)
                nc.sync.dma_start(wpjt[:], wpj_d[:])

            def qk_unit_gen(wt, ti, f, tt, th):
                """qk_unit split at op granularity so interleaving into B
                blocks never head-of-line-blocks a latency-critical op."""
                ps = psA.tile([128, 512], F32, tag="psa", name="psa")
                for kc in range(8):
                    nc.tensor.matmul(
                        ps[:],
                        wt[:, kc * 256 + f * 128: kc * 256 + f * 128 + 128],
                        xth[th][kc][:, tt * 512: tt * 512 + 512],
                        start=(kc == 0),
                        stop=(kc == 7),
                    )
                yield
                col = th * 1024 + tt * 512
                qs = qsp.tile([128, 512], BF16, tag="qs", name="qs")
                nc.scalar.copy(qs[:], ps[:])
                yield
                eng = nc.gpsimd if rope_flip[0] % 2 == 0 else nc.vector
                rope_flip[0] += 1
                qc = qcp.tile([128, 512], BF16, tag="qc", name="qc")
                eng.tensor_mul(qc[:], qs[:], ctab[:, col:col + 512])
                yield
                tm = tmp.tile([128, 512], BF16, tag="tm", name="tm")
                for ds, ss in SHUF:
                    nc.vector.tensor_mul(
                        tm[ds:ds + 32, :], qs[ss:ss + 32, :],
                        stabx[ss:ss + 32, col:col + 512],
                    )
                    yield
                eng.tensor_add(
                    proj[2 * ti + f][:, col:col + 512], qc[:], tm[:]
                )
                yield

            def v_unit_gen(wt, m, th):
                tg = th * 8 + m
                psv = psA.tile([128, 512], F32, tag="psa", name="psa")
                for kc in range(8):
                    nc.tensor.matmul(
                        psv[:, 0:256],
                        xth[th][kc][:, m * 128: m * 128 + 128],
                        wt[:, kc * 256: kc * 256 + 256],
                        start=(kc == 0),
                        stop=(kc == 7),
                    )
                yield
                nc.scalar.copy(
                    vt[tg // 4][:, (tg % 4) * 256: (tg % 4) * 256 + 256],
                    psv[:, 0:256],
                )
                yield

            def emit_A1():
                for tt in range(2):
                    for f in range(2):
                        for ti in range(4):
                            yield from qk_unit_gen(wts[ti], ti, f, tt, 1)
                    if tt == 0:
                        for m in range(4):
                            yield from v_unit_gen(wts[4], m, 1)
                for m in range(4, 8):
                    yield from v_unit_gen(wts[4], m, 1)

            def drain(gen, n):
                for _ in range(n):
                    try:
                        next(gen)
                    except StopIteration:
                        return False
                return True

            def scores_kk(qq, hp, kk):
                off = max(0, kk - 4 * qq) * 128
                qsl = slice(qq * 512 + off, qq * 512 + 512)
                ksl = slice(kk * 128, kk * 128 + 128)
                kT, qT = proj[2 + hp], proj[0 + hp]
                k2T, q2T = proj[6 + hp], proj[4 + hp]
                sps = [None, None]
                s2ps = [None, None]
                for hh in range(2):
                    rb = 64 * hh
                    sp = psS.tile([128, 512], F32, tag="sps", name="sps")
                    nc.tensor.matmul(
                        sp[:, off:512], kT[rb:rb + 64, ksl], qT[rb:rb + 64, qsl],
                        start=True, stop=True, tile_position=(rb, 0),
                    )
                    sps[hh] = sp
                for hh in range(2):
                    rb = 64 * hh
                    s2 = psS2.tile([128, 512], F32, tag="s2ps", name="s2ps")
                    nc.tensor.matmul(
                        s2[:, off:512], k2T[rb:rb + 64, ksl], q2T[rb:rb + 64, qsl],
                        start=True, stop=True, tile_position=(rb, 0),
                    )
                    s2ps[hh] = s2
                return sps, s2ps

            def bilinear_z(qq, hp, kk, sps, s2ps, zps, last):
                off = max(0, kk - 4 * qq) * 128
                for hh in range(2):
                    ts = tsp.tile([128, 512], F32, tag="ts", name="ts")
                    nc.scalar.copy(ts[:, off:512], sps[hh][:, off:512])
                    if kk >= 4 * qq:
                        j = kk - 4 * qq
                        nc.vector.tensor_mul(
                            ts[:, off:off + 128],
                            ts[:, off:off + 128],
                            masks[:, j * 512 + off: j * 512 + off + 128],
                        )
                    pt = ptp.tile([128, 512], BF16, tag="pt", name="pt")
                    nc.vector.tensor_mul(
                        pt[:, off:512], ts[:, off:512], s2ps[hh][:, off:512]
                    )
                    nc.tensor.matmul(
                        zps[64 * hh: 64 * hh + 64, off:512],
                        vt[kk // 4][:, (kk % 4) * 256 + (2 * hp + hh) * 64
                                    : (kk % 4) * 256 + (2 * hp + hh) * 64 + 64],
                        pt[:, off:512],
                        start=(kk == 0),
                        stop=(kk == last),
                    )

            def drain_z(qq, hp, zps):
                nc.scalar.copy(
                    zt[hp][:, qq * 512: qq * 512 + 512], zps[:]
                )

            def emit_B_gen(qq, hp, zpool, ztag):
                """Per-kk generator; zps holds both heads (hh at rows 64*hh)."""
                zps = zpool.tile([128, 512], F32, tag=ztag, name="zpg")
                last = 4 * qq + 3
                for kk in range(last + 1):
                    sps, s2ps = scores_kk(qq, hp, kk)
                    bilinear_z(qq, hp, kk, sps, s2ps, zps, last)
                    yield
                drain_z(qq, hp, zps)

            def emit_C_step(qq, ypool, ytag, tg, oo):
                yps = ypool.tile([128, 512], F32, tag=ytag, name="yps")
                for ci in range(2):
                    nc.tensor.matmul(
                        yps[:],
                        zt[ci][:, tg * 128: tg * 128 + 128],
                        wpjt[:, ci * 1024 + oo * 512
                             : ci * 1024 + oo * 512 + 512],
                        start=(ci == 0),
                        stop=(ci == 1),
                    )
                yo = ysb.tile([128, 512], BF16, tag="yo", name="yo")
                if (2 * tg + oo) % 2 == 0:
                    nc.scalar.copy(yo[:], yps[:])
                else:
                    nc.vector.tensor_copy(yo[:], yps[:])
                nc.sync.dma_start(
                    y_d[tg * 128: tg * 128 + 128, oo * 512: oo * 512 + 512],
                    yo[:],
                )

            def emit_C(qq, ypool, ytag):
                for tg in range(4 * qq, 4 * qq + 4):
                    for oo in range(2):
                        emit_C_step(qq, ypool, ytag, tg, oo)

            def emit_C_gen(qq, ypool, ytag):
                for tg in range(4 * qq, 4 * qq + 4):
                    for oo in range(2):
                        emit_C_step(qq, ypool, ytag, tg, oo)
                        yield

            # ---------------- emission schedule ----------------
            emit_A0()
            gen1 = emit_A1()

            def zip_blocks(qq, drip, extra=None, extra_every=0):
                ga = emit_B_gen(qq, 0, psZ, "zps")
                gb = emit_B_gen(qq, 1, psZ, "zps")
                rnd = 0
                alive = [ga, gb]
                while alive:
                    for g in list(alive):
                        try:
                            next(g)
                        except StopIteration:
                            alive.remove(g)
                    if drip:
                        drain(gen1, drip)
                    if extra is not None and extra_every and rnd % extra_every == extra_every - 1:
                        drain(extra, 1)
                    rnd += 1

            for qq in range(2):
                zip_blocks(qq, drip=3)
                for tg in range(4 * qq, 4 * qq + 4):
                    for oo in range(2):
                        emit_C_step(qq, psA, "psa", tg, oo)
                        drain(gen1, 2)
            zip_blocks(2, drip=4)
            drain(gen1, 1000)  # flush remaining A(1) units
            gc2 = emit_C_gen(2, psA, "psa")
            zip_blocks(3, drip=0, extra=gc2, extra_every=2)
            drain(gc2, 1000)
            emit_C(3, psA, "psa")

    nc.compile()
    return nc


def _get_program():
    global _PROGRAM
    if _PROGRAM is None:
        _PROGRAM = _build_program()
    return _PROGRAM


def _pack_w(ws):
    """ws: [256, 1024] (out-feature rows, in-feature cols) ->
    packed [128, 2048] where chunk kc, half f lives at cols
    [kc*256 + f*128, ...+128): lhsT tile = ws.T[kc*128:(kc+1)*128, f*128:...]."""
    a = np.ascontiguousarray(ws.T)  # [1024, 256]
    return np.ascontiguousarray(
        a.reshape(8, 128, 256).transpose(1, 0, 2).reshape(128, 2048)
    )


def _make_tabs():
    inv = 1.0 / (ROPE_BASE ** (np.arange(0, HEAD_DIM, 2, dtype=np.float32) / HEAD_DIM))
    t = np.arange(T, dtype=np.float32)
    ang = np.outer(t, inv)  # [T, 32]
    c32 = np.cos(ang).astype(np.float32).T  # [32, T]
    s32 = np.sin(ang).astype(np.float32).T
    ctab = np.tile(c32, (4, 1))  # [128, T]
    # source-aligned signed sin table: the shuffle op reads src rows ss:ss+32
    # of both the drained q tile and this table, writing dst rows ds:ds+32.
    # dst 0:32 needs +sin (src rows 32:64), dst 32:64 needs -sin (src 0:32).
    stabx = np.concatenate([-s32, s32, -s32, s32], axis=0)  # [128, T]
    tabs = np.ascontiguousarray(np.stack([ctab, stabx])).astype(NP_BF16)
    r = np.arange(128)[:, None]
    ccol = np.arange(512)[None, :]
    masks = np.ascontiguousarray(np.concatenate(
        [(ccol >= r + 128 * j).astype(np.float32) for j in range(4)], axis=1
    ))  # [128, 2048]
    return tabs, masks


def kernel(x, Wq, Wk, Wq2, Wk2, Wv, Wproj):
    x = np.asarray(x, dtype=np.float32)
    Wq = np.asarray(Wq, dtype=np.float32)
    Wk = np.asarray(Wk, dtype=np.float32)
    Wq2 = np.asarray(Wq2, dtype=np.float32)
    Wk2 = np.asarray(Wk2, dtype=np.float32)
    Wv = np.asarray(Wv, dtype=np.float32)
    Wproj = np.asarray(Wproj, dtype=np.float32)

    nc = _get_program()
    tabs, masks = _make_tabs()

    in_maps = []
    for c in range(N_CORES):
        b, g = divmod(c, HG)
        fsl = slice(g * FS, g * FS + FS)
        wpk = np.ascontiguousarray(
            np.stack(
                [
                    _pack_w(Wq[fsl] * (1.0 / HEAD_DIM)),
                    _pack_w(Wk[fsl]),
                    _pack_w(Wq2[fsl] * (1.0 / HEAD_DIM)),
                    _pack_w(Wk2[fsl]),
                    _pack_w(Wv[fsl]),
                ]
            )
        ).astype(NP_BF16)
        wpj = np.ascontiguousarray(
            Wproj[:, fsl].T.reshape(2, 128, 1024).transpose(1, 0, 2).reshape(128, 2048)
        ).astype(NP_BF16)
        xt = np.ascontiguousarray(x[b].T).astype(NP_BF16)
        in_maps.append({"xt": xt, "wpk": wpk, "wpj": wpj, "tabs": tabs, "masks": masks})

    res = run_bass_kernel_spmd(nc, in_maps, list(range(N_CORES))).results

    y = np.zeros((B, T, D_MODEL), dtype=np.float64)
    for c in range(N_CORES):
        b = c // HG
        y[b] += np.asarray(res[c]["y"]).astype(np.float64)
    return y.astype(np.float32)


# revision 9
# speedup vs baseline: 1.0569x; 1.0188x over previous
"""Causal bilinear self-attention kernel for 8 Trainium2 NeuronCores.

Sharding: core c handles batch b = c//4 and head group g = c%4 (4 of 16
heads, feature slice [256g, 256g+256)).  Each core computes its partial
output-projection contribution y_partial = z_slice @ Wproj[:, slice].T and
the host sums the 4 partials per batch.

The whole PE path runs in bfloat16 (x, weights, roped q/k, v, pattern, z,
y partials); PSUM accumulation stays fp32.  RoPE is applied on-chip: the
projection PSUM tile is drained to SBUF (Act), the sin-product of the
partner rows is formed by 4 partition-offset muls against a source-aligned
signed sin table (Pool), the cos-product on DVE, and the final add on Pool
writes the bf16 roped tile.  This removes the 4 sign-permuted-weight
projections entirely (~131k PE rows).

Schedule: phase A t-half 0 runs the q projection kc-major (4 concurrent
PSUM accumulators) so PE can start as soon as the first x tile lands;
t-half 1 is a generator interleaved into phase B.  Phase B blocks compute
scores S, S2 per head pair (2 row-tile_position matmuls each), the masked
bilinear product (Act copy + Pool mask + DVE mul), and accumulate z.  The
two independent blocks B(2,1)/B(3,0) are zipped to fill each other's
latency stalls.  Output projection C(qq) is interleaved per q-block.
"""

import numpy as np
import ml_dtypes

import concourse.tile as tile
from concourse import bacc, mybir
from concourse.bass_utils import run_bass_kernel_spmd

D_MODEL = 1024
N_HEAD = 16
HEAD_DIM = 64  # Dh
B, T = 2, 2048
ROPE_BASE = 10000.0
N_CORES = 8
HG = 4          # heads per core
FS = HG * HEAD_DIM  # 256 features per core

F32 = mybir.dt.float32
BF16 = mybir.dt.bfloat16
NP_BF16 = ml_dtypes.bfloat16

# (dst_row_start, src_row_start) pairs of the 32-row rope partner shuffle
SHUF = ((0, 32), (32, 0), (64, 96), (96, 64))

_PROGRAM = None


def _build_program():
    nc = bacc.Bacc("TRN2", target_bir_lowering=False, debug=False)

    xt_d = nc.dram_tensor("xt", [D_MODEL, T], BF16, kind="ExternalInput").ap()
    wpk_d = nc.dram_tensor("wpk", [5, 128, 2048], BF16, kind="ExternalInput").ap()
    wpj_d = nc.dram_tensor("wpj", [128, 2048], BF16, kind="ExternalInput").ap()
    tabs_d = nc.dram_tensor("tabs", [2, 128, 2048], BF16, kind="ExternalInput").ap()
    masks_d = nc.dram_tensor("masks", [128, 2048], F32, kind="ExternalInput").ap()
    y_d = nc.dram_tensor("y", [T, D_MODEL], BF16, kind="ExternalOutput").ap()

    with tile.TileContext(nc) as tc:
        with (
            tc.tile_pool(name="pers", bufs=1) as pers,
            tc.tile_pool(name="xp", bufs=16) as xp,
            tc.tile_pool(name="wp", bufs=5) as wp,
            tc.tile_pool(name="mkp", bufs=1) as mkp,
            tc.tile_pool(name="qsp", bufs=6) as qsp,
            tc.tile_pool(name="qcp", bufs=6) as qcp,
            tc.tile_pool(name="tmp", bufs=6) as tmp,
            tc.tile_pool(name="tsp", bufs=4) as tsp,
            tc.tile_pool(name="ptp", bufs=4) as ptp,
            tc.tile_pool(name="ysb", bufs=8) as ysb,
            tc.tile_pool(name="psA", bufs=2, space="PSUM") as psA,
            tc.tile_pool(name="psS", bufs=2, space="PSUM") as psS,
            tc.tile_pool(name="psS2", bufs=2, space="PSUM") as psS2,
            tc.tile_pool(name="psZ", bufs=2, space="PSUM") as psZ,
        ):
            # persistent tiles
            proj = [pers.tile([128, T], BF16, tag=f"proj{i}", name=f"proj{i}")
                    for i in range(8)]
            # proj[2*ti+f] = chunk f of tensor ti (0=q,1=k,2=q2,3=k2)
            vt = [pers.tile([128, 1024], BF16, tag=f"v{i}", name=f"v{i}")
                  for i in range(4)]
            zt = [pers.tile([128, T], BF16, tag=f"z{i}", name=f"z{i}")
                  for i in range(2)]
            ctab = pers.tile([128, 2048], BF16, tag="ctab")
            stabx = pers.tile([128, 2048], BF16, tag="stabx")
            masks = mkp.tile([128, 2048], F32, tag="masks")
            wpjt = pers.tile([128, 2048], BF16, tag="wpjt")

            wts = []       # 5 resident weight tiles (q,k,q2,k2,v)
            xth = [[], []]  # x tiles per t-half

            rope_flip = [0]

            def rope_consume(ps, ti, f, col):
                """Drain + RoPE a [128,512] projection PSUM tile into
                proj[2*ti+f][:, col:col+512].  The cos-mul and final add
                alternate between Pool and DVE so neither queue backs up."""
                qs = qsp.tile([128, 512], BF16, tag="qs", name="qs")
                nc.scalar.copy(qs[:], ps[:])
                eng = nc.vector if 12 <= rope_flip[0] < 16 else nc.gpsimd
                rope_flip[0] += 1
                qc = qcp.tile([128, 512], BF16, tag="qc", name="qc")
                eng.tensor_mul(qc[:], qs[:], ctab[:, col:col + 512])
                tm = tmp.tile([128, 512], BF16, tag="tm", name="tm")
                for ds, ss in SHUF:
                    nc.vector.tensor_mul(
                        tm[ds:ds + 32, :], qs[ss:ss + 32, :],
                        stabx[ss:ss + 32, col:col + 512],
                    )
                eng.tensor_add(
                    proj[2 * ti + f][:, col:col + 512], qc[:], tm[:]
                )

            def qk_unit(wt, ti, f, tt, th):
                """One standard projection unit (8-deep kc chain)."""
                ps = psA.tile([128, 512], F32, tag="psa", name="psa")
                for kc in range(8):
                    nc.tensor.matmul(
                        ps[:],
                        wt[:, kc * 256 + f * 128: kc * 256 + f * 128 + 128],
                        xth[th][kc][:, tt * 512: tt * 512 + 512],
                        start=(kc == 0),
                        stop=(kc == 7),
                    )
                rope_consume(ps, ti, f, th * 1024 + tt * 512)

            def v_unit(wt, m, th):
                tg = th * 8 + m
                psv = psA.tile([128, 512], F32, tag="psa", name="psa")
                for kc in range(8):
                    nc.tensor.matmul(
                        psv[:, 0:256],
                        xth[th][kc][:, m * 128: m * 128 + 128],
                        wt[:, kc * 256: kc * 256 + 256],
                        start=(kc == 0),
                        stop=(kc == 7),
                    )
                nc.scalar.copy(
                    vt[tg // 4][:, (tg % 4) * 256: (tg % 4) * 256 + 256],
                    psv[:, 0:256],
                )

            def emit_A0():
                # q weights first, then x t-half 0, tabs, remaining weights
                wq = wp.tile([128, 2048], BF16, tag="wt", name="wt")
                nc.sync.dma_start(wq[:, 0:1024], wpk_d[0][:, 0:1024])
                wts.append(wq)
                for kc in range(8):
                    xtl = xp.tile([128, 1024], BF16, tag="xt", name="xtl")
                    nc.sync.dma_start(
                        xtl[:], xt_d[kc * 128: kc * 128 + 128, 0:1024]
                    )
                    xth[0].append(xtl)
                    if kc == 0:
                        nc.sync.dma_start(wq[:, 1024:2048], wpk_d[0][:, 1024:2048])
                for wi in range(1, 5):
                    wt = wp.tile([128, 2048], BF16, tag="wt", name="wt")
                    nc.sync.dma_start(wt[:], wpk_d[wi])
                    wts.append(wt)
                nc.sync.dma_start(ctab[:], tabs_d[0])
                nc.sync.dma_start(stabx[:], tabs_d[1])
                nc.sync.dma_start(masks[:], masks_d)

                # kc-major q projection: 4 concurrent accumulators so the PE
                # starts on x tile 0 instead of waiting for the full t-half
                qpools = [(psA, "psa"), (psS, "sps"), (psS2, "s2ps"), (psZ, "zps")]
                units = [(0, 0), (1, 0), (0, 1), (1, 1)]  # (f, tt)
                qps = [pool.tile([128, 512], F32, tag=tag, name="qps")
                       for pool, tag in qpools]
                for kc in range(8):
                    for u, (f, tt) in enumerate(units):
                        nc.tensor.matmul(
                            qps[u][:],
                            wq[:, kc * 256 + f * 128: kc * 256 + f * 128 + 128],
                            xth[0][kc][:, tt * 512: tt * 512 + 512],
                            start=(kc == 0),
                            stop=(kc == 7),
                        )
                # (tt, f)-major so each B block's proj dependencies clear the
                # Pool/Act queues in block order
                for u, (f, tt) in enumerate(units):
                    rope_consume(qps[u], 0, f, tt * 512)
                    for ti in range(1, 4):
                        qk_unit(wts[ti], ti, f, tt, 0)
                for m in range(8):
                    v_unit(wts[4], m, 0)

                # queue t-half-1 x and the output weight behind phase-A DMA
                for kc in range(8):
                    xtl = xp.tile([128, 1024], BF16, tag="xt", name="xtl")
                    nc.sync.dma_start(
                        xtl[:], xt_d[kc * 128: kc * 128 + 128, 1024:2048]
                    )
                    xth[1].append(xtl)
                nc.sync.dma_start(wpjt[:], wpj_d[:])

            def qk_unit_gen(wt, ti, f, tt, th):
                """qk_unit split at op granularity so interleaving into B
                blocks never head-of-line-blocks a latency-critical op."""
                ps = psA.tile([128, 512], F32, tag="psa", name="psa")
                for kc in range(8):
                    nc.tensor.matmul(
                        ps[:],
                        wt[:, kc * 256 + f * 128: kc * 256 + f * 128 + 128],
                        xth[th][kc][:, tt * 512: tt * 512 + 512],
                        start=(kc == 0),
                        stop=(kc == 7),
                    )
                yield
                col = th * 1024 + tt * 512
                qs = qsp.tile([128, 512], BF16, tag="qs", name="qs")
                nc.scalar.copy(qs[:], ps[:])
                yield
                eng = nc.gpsimd
                qc = qcp.tile([128, 512], BF16, tag="qc", name="qc")
                eng.tensor_mul(qc[:], qs[:], ctab[:, col:col + 512])
                yield
                tm = tmp.tile([128, 512], BF16, tag="tm", name="tm")
                for ds, ss in SHUF:
                    nc.vector.tensor_mul(
                        tm[ds:ds + 32, :], qs[ss:ss + 32, :],
                        stabx[ss:ss + 32, col:col + 512],
                    )
                    yield
                eng.tensor_add(
                    proj[2 * ti + f][:, col:col + 512], qc[:], tm[:]
                )
                yield

            def v_unit_gen(wt, m, th):
                tg = th * 8 + m
                psv = psA.tile([128, 512], F32, tag="psa", name="psa")
                for kc in range(8):
                    nc.tensor.matmul(
                        psv[:, 0:256],
                        xth[th][kc][:, m * 128: m * 128 + 128],
                        wt[:, kc * 256: kc * 256 + 256],
                        start=(kc == 0),
                        stop=(kc == 7),
                    )
                yield
                nc.scalar.copy(
                    vt[tg // 4][:, (tg % 4) * 256: (tg % 4) * 256 + 256],
                    psv[:, 0:256],
                )
                yield

            def emit_A1():
                for tt in range(2):
                    for f in range(2):
                        for ti in range(4):
                            yield from qk_unit_gen(wts[ti], ti, f, tt, 1)
                    if tt == 0:
                        for m in range(4):
                            yield from v_unit_gen(wts[4], m, 1)
                for m in range(4, 8):
                    yield from v_unit_gen(wts[4], m, 1)

            def drain(gen, n):
                for _ in range(n):
                    try:
                        next(gen)
                    except StopIteration:
                        return False
                return True

            def scores_kk(qq, hp, kk):
                off = max(0, kk - 4 * qq) * 128
                qsl = slice(qq * 512 + off, qq * 512 + 512)
                ksl = slice(kk * 128, kk * 128 + 128)
                kT, qT = proj[2 + hp], proj[0 + hp]
                k2T, q2T = proj[6 + hp], proj[4 + hp]
                sps = [None, None]
                s2ps = [None, None]
                for hh in range(2):
                    rb = 64 * hh
                    sp = psS.tile([128, 512], F32, tag="sps", name="sps")
                    nc.tensor.matmul(
                        sp[:, off:512], kT[rb:rb + 64, ksl], qT[rb:rb + 64, qsl],
                        start=True, stop=True, tile_position=(rb, 0),
                    )
                    sps[hh] = sp
                for hh in range(2):
                    rb = 64 * hh
                    s2 = psS2.tile([128, 512], F32, tag="s2ps", name="s2ps")
                    nc.tensor.matmul(
                        s2[:, off:512], k2T[rb:rb + 64, ksl], q2T[rb:rb + 64, qsl],
                        start=True, stop=True, tile_position=(rb, 0),
                    )
                    s2ps[hh] = s2
                return sps, s2ps

            def bilinear_z(qq, hp, kk, sps, s2ps, zps, last):
                off = max(0, kk - 4 * qq) * 128
                for hh in range(2):
                    ts = tsp.tile([128, 512], F32, tag="ts", name="ts")
                    nc.scalar.copy(ts[:, off:512], sps[hh][:, off:512])
                    if kk >= 4 * qq:
                        j = kk - 4 * qq
                        nc.gpsimd.tensor_mul(
                            ts[:, off:off + 128],
                            ts[:, off:off + 128],
                            masks[:, j * 512 + off: j * 512 + off + 128],
                        )
                    pt = ptp.tile([128, 512], BF16, tag="pt", name="pt")
                    nc.vector.tensor_mul(
                        pt[:, off:512], ts[:, off:512], s2ps[hh][:, off:512]
                    )
                    nc.tensor.matmul(
                        zps[64 * hh: 64 * hh + 64, off:512],
                        vt[kk // 4][:, (kk % 4) * 256 + (2 * hp + hh) * 64
                                    : (kk % 4) * 256 + (2 * hp + hh) * 64 + 64],
                        pt[:, off:512],
                        start=(kk == 0),
                        stop=(kk == last),
                    )

            def drain_z(qq, hp, zps):
                nc.scalar.copy(
                    zt[hp][:, qq * 512: qq * 512 + 512], zps[:]
                )

            def emit_B_gen(qq, hp, zpool, ztag):
                """Per-kk generator; zps holds both heads (hh at rows 64*hh)."""
                zps = zpool.tile([128, 512], F32, tag=ztag, name="zpg")
                last = 4 * qq + 3
                for kk in range(last + 1):
                    sps, s2ps = scores_kk(qq, hp, kk)
                    bilinear_z(qq, hp, kk, sps, s2ps, zps, last)
                    yield
                drain_z(qq, hp, zps)

            def emit_C_step(qq, ypool, ytag, tg, oo):
                yps = ypool.tile([128, 512], F32, tag=ytag, name="yps")
                for ci in range(2):
                    nc.tensor.matmul(
                        yps[:],
                        zt[ci][:, tg * 128: tg * 128 + 128],
                        wpjt[:, ci * 1024 + oo * 512
                             : ci * 1024 + oo * 512 + 512],
                        start=(ci == 0),
                        stop=(ci == 1),
                    )
                yo = ysb.tile([128, 512], BF16, tag="yo", name="yo")
                if (2 * tg + oo) % 2 == 0:
                    nc.scalar.copy(yo[:], yps[:])
                else:
                    nc.vector.tensor_copy(yo[:], yps[:])
                nc.sync.dma_start(
                    y_d[tg * 128: tg * 128 + 128, oo * 512: oo * 512 + 512],
                    yo[:],
                )

            def emit_C(qq, ypool, ytag):
                for tg in range(4 * qq, 4 * qq + 4):
                    for oo in range(2):
                        emit_C_step(qq, ypool, ytag, tg, oo)

            def emit_C_gen(qq, ypool, ytag):
                for tg in range(4 * qq, 4 * qq + 4):
                    for oo in range(2):
                        emit_C_step(qq, ypool, ytag, tg, oo)
                        yield

            # ---------------- emission schedule ----------------
            emit_A0()
            gen1 = emit_A1()

            def zip_blocks(qq, drip, extra=None, extra_every=0):
                ga = emit_B_gen(qq, 0, psZ, "zps")
                gb = emit_B_gen(qq, 1, psZ, "zps")
                rnd = 0
                alive = [ga, gb]
                while alive:
                    for g in list(alive):
                        try:
                            next(g)
                        except StopIteration:
                            alive.remove(g)
                    if drip:
                        drain(gen1, drip)
                    if extra is not None and extra_every and rnd % extra_every == extra_every - 1:
                        drain(extra, 1)
                    rnd += 1

            for qq in range(2):
                zip_blocks(qq, drip=3)
                for tg in range(4 * qq, 4 * qq + 4):
                    for oo in range(2):
                        emit_C_step(qq, psA, "psa", tg, oo)
                        drain(gen1, 2)
            zip_blocks(2, drip=4)
            drain(gen1, 1000)  # flush remaining A(1) units
            gc2 = emit_C_gen(2, psA, "psa")
            zip_blocks(3, drip=0, extra=gc2, extra_every=2)
            drain(gc2, 1000)
            emit_C(3, psA, "psa")

    nc.compile()
    return nc


def _get_program():
    global _PROGRAM
    if _PROGRAM is None:
        _PROGRAM = _build_program()
    return _PROGRAM


def _pack_w(ws):
    """ws: [256, 1024] (out-feature rows, in-feature cols) ->
    packed [128, 2048] where chunk kc, half f lives at cols
    [kc*256 + f*128, ...+128): lhsT tile = ws.T[kc*128:(kc+1)*128, f*128:...]."""
    a = np.ascontiguousarray(ws.T)  # [1024, 256]
    return np.ascontiguousarray(
        a.reshape(8, 128, 256).transpose(1, 0, 2).reshape(128, 2048)
    )


def _make_tabs():
    inv = 1.0 / (ROPE_BASE ** (np.arange(0, HEAD_DIM, 2, dtype=np.float32) / HEAD_DIM))
    t = np.arange(T, dtype=np.float32)
    ang = np.outer(t, inv)  # [T, 32]
    c32 = np.cos(ang).astype(np.float32).T  # [32, T]
    s32 = np.sin(ang).astype(np.float32).T
    ctab = np.tile(c32, (4, 1))  # [128, T]
    # source-aligned signed sin table: the shuffle op reads src rows ss:ss+32
    # of both the drained q tile and this table, writing dst rows ds:ds+32.
    # dst 0:32 needs +sin (src rows 32:64), dst 32:64 needs -sin (src 0:32).
    stabx = np.concatenate([-s32, s32, -s32, s32], axis=0)  # [128, T]
    tabs = np.ascontiguousarray(np.stack([ctab, stabx])).astype(NP_BF16)
    r = np.arange(128)[:, None]
    ccol = np.arange(512)[None, :]
    masks = np.ascontiguousarray(np.concatenate(
        [(ccol >= r + 128 * j).astype(np.float32) for j in range(4)], axis=1
    ))  # [128, 2048]
    return tabs, masks


def kernel(x, Wq, Wk, Wq2, Wk2, Wv, Wproj):
    x = np.asarray(x, dtype=np.float32)
    Wq = np.asarray(Wq, dtype=np.float32)
    Wk = np.asarray(Wk, dtype=np.float32)
    Wq2 = np.asarray(Wq2, dtype=np.float32)
    Wk2 = np.asarray(Wk2, dtype=np.float32)
    Wv = np.asarray(Wv, dtype=np.float32)
    Wproj = np.asarray(Wproj, dtype=np.float32)

    nc = _get_program()
    tabs, masks = _make_tabs()

    in_maps = []
    for c in range(N_CORES):
        b, g = divmod(c, HG)
        fsl = slice(g * FS, g * FS + FS)
        wpk = np.ascontiguousarray(
            np.stack(
                [
                    _pack_w(Wq[fsl] * (1.0 / HEAD_DIM)),
                    _pack_w(Wk[fsl]),
                    _pack_w(Wq2[fsl] * (1.0 / HEAD_DIM)),
                    _pack_w(Wk2[fsl]),
                    _pack_w(Wv[fsl]),
                ]
            )
        ).astype(NP_BF16)
        wpj = np.ascontiguousarray(
            Wproj[:, fsl].T.reshape(2, 128, 1024).transpose(1, 0, 2).reshape(128, 2048)
        ).astype(NP_BF16)
        xt = np.ascontiguousarray(x[b].T).astype(NP_BF16)
        in_maps.append({"xt": xt, "wpk": wpk, "wpj": wpj, "tabs": tabs, "masks": masks})

    res = run_bass_kernel_spmd(nc, in_maps, list(range(N_CORES))).results

    y = np.zeros((B, T, D_MODEL), dtype=np.float64)
    for c in range(N_CORES):
        b = c // HG
        y[b] += np.asarray(res[c]["y"]).astype(np.float64)
    return y.astype(np.float32)
